# revision 33
# baseline (speedup 1.0000x reference)
"""Trainium2 Bass kernel for ConditionalEdgeDenoiser (GNN edge MLP denoiser).

Reference computation (per batch b, nodes i,j):
    h = concat([edge_t[b,i,j,:],            # 4   (EC)
                node_ctx[b,i,:],            # 80  (src = x_cond||code_cond)
                node_ctx[b,j,:],            # 80  (dst)
                time_emb[b,:]])             # 128 (TDIM)
    h1 = silu(h @ W1 + b1); h2 = silu(h1 @ W2 + b2); out = (h2 @ W3 + b3) * mask

Strategy (8 cores, data-parallel over (B x row-halves) = 8 shards of 128 rows):
  * Activations live as [hid on partitions, edge-columns on free dim]; a tile
    covers RPT=4 grid rows x 256 cols = 1024 edges.
  * Layer 1 is ONE augmented matmul per hid-half: the stationary operand
    stacks [W1_edge (4) ; W1_dst (80) ; srcbias rows (RPT)] and the moving
    operand stacks [edge_T (4) ; node_ctx_T (80) ; row-indicators (RPT)].
    srcbias = node_ctx[i] @ W1_src + time_emb @ W1_time + b1 is precomputed
    on-device (transposed, per row i) and DMA'd into the stationary tile per
    tile iteration.  So PSUM holds the complete pre-activation and ScalarE
    does pure Silu over big contiguous tiles.
  * All matmul inputs are float32r (TF32-like, 1 row/cycle at N=512 vs 4x
    slower plain fp32); PSUM accumulates fp32.
  * The emission order is a 3-stage software pipeline (L1+silu1 for tile k,
    L2+silu2 for k-1, L3+mask+out for k-2) so each engine's static
    instruction order interleaves adjacent tiles — ScalarE (the bottleneck
    engine) runs back-to-back.
"""

import os
import sys

sys.path.insert(0, "/opt/trn_rl_repo")
os.environ.setdefault("MYCRO_LOCAL_CACHE", "1")

import numpy as np

import concourse.bass as bass  # noqa: E402
import concourse.mybir as mybir  # noqa: E402
import concourse.tile as tile  # noqa: E402
from concourse import bacc  # noqa: E402
from concourse.bass_utils import run_bass_kernel_spmd  # noqa: E402

B, N, EC, FEAT, CODE, HID, TDIM = 4, 256, 4, 64, 16, 256, 128
NCTX = FEAT + CODE  # 80
NCORES = 8
RPT = 4                      # grid rows per tile
E = RPT * N                  # 1024 edge columns per tile
CH = 512                     # matmul moving-dim chunk (fp32 PSUM bank limit)
NCH = E // CH                # chunks per tile
ROWS = N // 2                # 128 grid rows per core
NT = ROWS // RPT             # 32 tiles per core
KAUG = EC + NCTX + RPT       # 88 = augmented contraction dim for layer 1

F32 = mybir.dt.float32
F32R = mybir.dt.float32r
AF = mybir.ActivationFunctionType
ALU = mybir.AluOpType

_CACHE = {}


def _build():
    nc = bacc.Bacc("TRN2", debug=False, num_devices=NCORES)

    # ---- DRAM I/O (per core) ----
    edge_d = nc.dram_tensor("edge", [NT, EC, E], F32R, kind="ExternalInput")
    nctxT_d = nc.dram_tensor("nctxT", [NCTX, N], F32R, kind="ExternalInput")
    srcb_d = nc.dram_tensor("srcb", [ROWS, HID], F32R, kind="ExternalInput")
    mask4_d = nc.dram_tensor("mask4", [EC, E], F32, kind="ExternalInput")
    mif4_d = nc.dram_tensor("mif4", [EC, ROWS], F32, kind="ExternalInput")
    w1ed_d = nc.dram_tensor("w1ed", [EC + NCTX, HID], F32R, kind="ExternalInput")
    b2c_d = nc.dram_tensor("b2c", [128, 2], F32, kind="ExternalInput")
    b3_d = nc.dram_tensor("b3", [EC], F32, kind="ExternalInput")
    w2_d = nc.dram_tensor("w2", [HID, HID], F32R, kind="ExternalInput")
    w3_d = nc.dram_tensor("w3", [HID, EC], F32R, kind="ExternalInput")
    rhsstat_d = nc.dram_tensor("rhsstat", [NCTX + RPT, E], F32R, kind="ExternalInput")
    out_d = nc.dram_tensor("out", [NT, EC, E], F32, kind="ExternalOutput")

    with tile.TileContext(nc) as tc:
        with tc.tile_pool(name="const", bufs=1) as cp, \
             tc.tile_pool(name="h", bufs=2) as hp, \
             tc.tile_pool(name="o", bufs=3) as op, \
             tc.tile_pool(name="ps", bufs=1, space="PSUM") as pp:

            # ---------- augmented layer-1 operands (ping-pong pairs) ----------
            # lh[q]: [KAUG, 256] stationary tile, halves at cols 0:128 / 128:256.
            # rhs_t[q]: [KAUG, E] moving tile.  These feed tile 0, so their
            # loads come first and are merged into one DMA per buffer.
            lh = [None, None]
            rhs_t = [None, None]
            for q in range(2):
                lt = cp.tile([KAUG, HID], F32R, tag=f"lh{q}")
                nc.sync.dma_start(out=lt[0:EC + NCTX, :], in_=w1ed_d[:])
                lh[q] = lt
                rt = cp.tile([KAUG, E], F32R, tag=f"rhs{q}")
                nc.sync.dma_start(out=rt[EC:KAUG, :], in_=rhsstat_d[:])
                rhs_t[q] = rt

            # srcbT[i, h] = node_ctx[i] @ W1s + temb @ W1t + b1 (host-prepared)
            srcbT = cp.tile([ROWS, HID], F32R, tag="srcbT")
            nc.gpsimd.dma_start(out=srcbT, in_=srcb_d[:])

            # ---------- constant loads (Pool/SWDGE: off the HWDGE queue) ----------
            w2k0 = cp.tile([128, HID], F32R, tag="w2k0")
            nc.gpsimd.dma_start(out=w2k0, in_=w2_d[0:128])
            w2k1 = cp.tile([128, HID], F32R, tag="w2k1")
            nc.gpsimd.dma_start(out=w2k1, in_=w2_d[128:256])
            w30 = cp.tile([128, EC], F32R, tag="w30")
            nc.gpsimd.dma_start(out=w30, in_=w3_d[0:128])
            w31 = cp.tile([128, EC], F32R, tag="w31")
            nc.gpsimd.dma_start(out=w31, in_=w3_d[128:256])
            b2c = cp.tile([128, 2], F32, tag="b2c")
            nc.gpsimd.dma_start(out=b2c, in_=b2c_d[:])
            b3c = cp.tile([EC, 1], F32, tag="b3c")
            nc.gpsimd.dma_start(out=b3c, in_=b3_d[:].rearrange("(p o) -> p o", o=1))

            # mask tiles (host-prepared): mask4[c, r*N+j] = mask_j; mif4[c, i] = mask_i
            mask4 = cp.tile([EC, E], F32, tag="mask4")
            nc.sync.dma_start(out=mask4, in_=mask4_d[:])
            mif4 = cp.tile([EC, ROWS], F32, tag="mif4")
            nc.sync.dma_start(out=mif4, in_=mif4_d[:])

            # ---------- main loop: 3-stage software pipeline ----------
            h1s, h2s, ots = {}, {}, {}
            for k in range(NT + 2):
                if k < NT:
                    rhs = rhs_t[k % 2]
                    nc.sync.dma_start(out=rhs[0:EC, :], in_=edge_d[k])
                    lht = lh[k % 2]
                    # per-tile srcbias rows -> stationary tile (SWDGE, Pool)
                    nc.gpsimd.dma_start(out=lht[EC + NCTX:KAUG, :],
                                        in_=srcbT[RPT * k:RPT * (k + 1), :])
                    h1 = hp.tile([128, 2 * E], F32R, tag="h1")
                    p1 = pp.tile([128, 2 * E], F32, name=f"p1_{k}", tag="p1")
                    for h in range(2):
                        for c in range(NCH):
                            nc.tensor.matmul(
                                p1[:, h * E + c * CH:h * E + (c + 1) * CH],
                                lhsT=lht[:, h * 128:(h + 1) * 128],
                                rhs=rhs[:, c * CH:(c + 1) * CH],
                                start=True, stop=True)
                    nc.scalar.activation(h1, p1, AF.Silu)
                    h1s[k] = h1

                if 1 <= k <= NT:
                    j = k - 1
                    h1 = h1s.pop(j)
                    h2 = hp.tile([128, 2 * E], F32R, tag="h2")
                    for h in range(2):
                        # per-half PSUM tile: silu2 of half a releases its slot
                        # before half b finishes, loosening the L2(j+1) gate
                        p2h = pp.tile([128, E], F32, name=f"p2_{j}_{h}",
                                      tag=f"p2{'ab'[h]}")
                        for c in range(NCH):
                            dst = p2h[:, c * CH:(c + 1) * CH]
                            nc.tensor.matmul(
                                dst, lhsT=w2k0[:, h * 128:(h + 1) * 128],
                                rhs=h1[:, c * CH:(c + 1) * CH],
                                start=True, stop=False)
                            nc.tensor.matmul(
                                dst, lhsT=w2k1[:, h * 128:(h + 1) * 128],
                                rhs=h1[:, E + c * CH:E + (c + 1) * CH],
                                start=False, stop=True)
                        nc.scalar.activation(h2[:, h * E:(h + 1) * E], p2h, AF.Silu,
                                             bias=b2c[:, h:h + 1])
                    h2s[j] = h2

                if k >= 2:
                    i = k - 2
                    h2 = h2s.pop(i)
                    p3 = pp.tile([EC, E], F32, name=f"p3_{i}", tag="p2a")
                    for c in range(NCH):
                        dst = p3[:, c * CH:(c + 1) * CH]
                        nc.tensor.matmul(dst, lhsT=w30,
                                         rhs=h2[:, c * CH:(c + 1) * CH],
                                         start=True, stop=False)
                        nc.tensor.matmul(dst, lhsT=w31,
                                         rhs=h2[:, E + c * CH:E + (c + 1) * CH],
                                         start=False, stop=True)
                    ot = op.tile([EC, E], F32, name=f"ot{i}", tag="ot")
                    # one big op reads PSUM (releases the p13 slot early)
                    nc.vector.tensor_scalar_add(out=ot, in0=p3, scalar1=b3c)
                    nc.vector.tensor_mul(out=ot, in0=ot, in1=mask4)  # * mask_j
                    for r in range(RPT):
                        # * mask_i for this grid row
                        nc.vector.tensor_scalar_mul(
                            out=ot[:, r * N:(r + 1) * N],
                            in0=ot[:, r * N:(r + 1) * N],
                            scalar1=mif4[:, RPT * i + r:RPT * i + r + 1])
                    nc.sync.dma_start(out=out_d[i], in_=ot)

    nc.compile()
    return nc


def _get_nc():
    if "nc" not in _CACHE:
        _CACHE["nc"] = _build()
    return _CACHE["nc"]


def _time_embedding(t):
    half = TDIM // 2
    freqs = np.exp(-np.arange(half, dtype=np.float32)
                   * (np.float32(np.log(10000.0)) / np.float32(half - 1)))
    args = np.asarray(t).astype(np.float32)[:, None] * freqs[None, :]
    return np.concatenate([np.sin(args), np.cos(args)], axis=1).astype(np.float32)


def _indicator():
    ind = np.zeros((RPT, E), dtype=np.float32)
    for r in range(RPT):
        ind[r, r * N:(r + 1) * N] = 1.0
    return ind


def _prepare_in_maps(edge_t, x_cond, code_cond, t, node_mask, W1, b1, W2, b2, W3, b3):
    edge_t = np.ascontiguousarray(np.asarray(edge_t, dtype=np.float32))
    node_ctx = np.concatenate(
        [np.asarray(x_cond, np.float32), np.asarray(code_cond, np.float32)], axis=-1)
    temb = _time_embedding(t)                       # [B, TDIM]
    maskf = np.asarray(node_mask).astype(np.float32)  # [B, N]
    W1 = np.asarray(W1, np.float32)
    w1e = np.ascontiguousarray(W1[0:EC])
    w1s = W1[EC:EC + NCTX]
    w1d = np.ascontiguousarray(W1[EC + NCTX:EC + 2 * NCTX])
    w1t = W1[EC + 2 * NCTX:]
    b1 = np.asarray(b1, np.float32)
    b2c = np.ascontiguousarray(np.asarray(b2, np.float32).reshape(2, 128).T)
    b3 = np.asarray(b3, np.float32)
    W2 = np.ascontiguousarray(np.asarray(W2, np.float32))
    W3 = np.ascontiguousarray(np.asarray(W3, np.float32))
    # srcbias (bias precomputation — 0.1% of model FLOPs): [B*N, HID]
    srcb_full = (node_ctx.reshape(B * N, NCTX) @ w1s
                 + (temb @ w1t + b1)[:, None, :].repeat(N, axis=1).reshape(B * N, HID)
                 ).astype(np.float32)

    in_maps = []
    for c in range(NCORES):
        b, ih = c // 2, c % 2
        i0 = ih * ROWS
        es = edge_t[b, i0:i0 + ROWS]               # [ROWS, N, EC]
        er = np.ascontiguousarray(
            es.reshape(NT, RPT, N, EC).transpose(0, 3, 1, 2).reshape(NT, EC, E))
        in_maps.append({
            "edge": er,
            "nctxT": np.ascontiguousarray(node_ctx[b].T),
            "srcb": np.ascontiguousarray(srcb_full[b * N + i0:b * N + i0 + ROWS]),
            "mask4": np.ascontiguousarray(np.tile(maskf[b], (EC, RPT))),
            "mif4": np.ascontiguousarray(
                np.tile(maskf[b, i0:i0 + ROWS], (EC, 1))),
            "w1ed": np.ascontiguousarray(np.vstack([w1e, w1d])),
            "b2c": b2c, "b3": b3, "w2": W2, "w3": W3,
            "rhsstat": np.ascontiguousarray(
                np.vstack([np.tile(node_ctx[b].T, (1, RPT)), _indicator()])),
        })
    return in_maps


def _assemble(results):
    out = np.empty((B, N, N, EC), dtype=np.float32)
    for c in range(NCORES):
        b, ih = c // 2, c % 2
        i0 = ih * ROWS
        o = results[c]["out"]                      # [NT, EC, E]
        out[b, i0:i0 + ROWS] = (
            o.reshape(NT, EC, RPT, N).transpose(0, 2, 3, 1).reshape(ROWS, N, EC))
    return out


def _run(in_maps, trace=False, **kwargs):
    nc = _get_nc()
    return run_bass_kernel_spmd(nc, in_maps, list(range(NCORES)), trace=trace, **kwargs)


def kernel(**inputs):
    in_maps = _prepare_in_maps(**inputs)
    res = _run(in_maps)
    return _assemble(res.results)


# revision 34
# speedup vs baseline: 1.0893x; 1.0893x over previous
"""Trainium2 Bass kernel for ConditionalEdgeDenoiser (GNN edge MLP denoiser).

Reference computation (per batch b, nodes i,j):
    h = concat([edge_t[b,i,j,:],            # 4   (EC)
                node_ctx[b,i,:],            # 80  (src = x_cond||code_cond)
                node_ctx[b,j,:],            # 80  (dst)
                time_emb[b,:]])             # 128 (TDIM)
    h1 = silu(h @ W1 + b1); h2 = silu(h1 @ W2 + b2); out = (h2 @ W3 + b3) * mask

Strategy (8 cores, data-parallel over (B x row-halves) = 8 shards of 128 rows):
  * Activations live as [hid on partitions, edge-columns on free dim]; a tile
    covers RPT=4 grid rows x 256 cols = 1024 edges.
  * Layer 1 is ONE augmented matmul per hid-half: the stationary operand
    stacks [W1_edge (4) ; W1_dst (80) ; srcbias rows (RPT)] and the moving
    operand stacks [edge_T (4) ; node_ctx_T (80) ; row-indicators (RPT)].
    srcbias = node_ctx[i] @ W1_src + time_emb @ W1_time + b1 is precomputed
    on-device (transposed, per row i) and DMA'd into the stationary tile per
    tile iteration.  So PSUM holds the complete pre-activation and ScalarE
    does pure Silu over big contiguous tiles.
  * All matmul inputs are float32r (TF32-like, 1 row/cycle at N=512 vs 4x
    slower plain fp32); PSUM accumulates fp32.
  * The emission order is a 3-stage software pipeline (L1+silu1 for tile k,
    L2+silu2 for k-1, L3+mask+out for k-2) so each engine's static
    instruction order interleaves adjacent tiles — ScalarE (the bottleneck
    engine) runs back-to-back.
"""

import os
import sys

sys.path.insert(0, "/opt/trn_rl_repo")
os.environ.setdefault("MYCRO_LOCAL_CACHE", "1")

import numpy as np

import concourse.bass as bass  # noqa: E402
import concourse.mybir as mybir  # noqa: E402
import concourse.tile as tile  # noqa: E402
from concourse import bacc  # noqa: E402
from concourse.bass_utils import run_bass_kernel_spmd  # noqa: E402

B, N, EC, FEAT, CODE, HID, TDIM = 4, 256, 4, 64, 16, 256, 128
NCTX = FEAT + CODE  # 80
NCORES = 8
RPT = 4                      # grid rows per tile
E = RPT * N                  # 1024 edge columns per tile
CH = 512                     # matmul moving-dim chunk (fp32 PSUM bank limit)
NCH = E // CH                # chunks per tile
ROWS = N // 2                # 128 grid rows per core
NT = ROWS // RPT             # 32 tiles per core
KAUG = EC + NCTX + RPT       # 88 = augmented contraction dim for layer 1

F32 = mybir.dt.float32
F32R = mybir.dt.float32r
AF = mybir.ActivationFunctionType
ALU = mybir.AluOpType

_CACHE = {}


def _build():
    nc = bacc.Bacc("TRN2", debug=False, num_devices=NCORES)

    # ---- DRAM I/O (per core) ----
    edge_d = nc.dram_tensor("edge", [NT, EC, E], F32R, kind="ExternalInput")
    nctxT_d = nc.dram_tensor("nctxT", [NCTX, N], F32R, kind="ExternalInput")
    srcb_d = nc.dram_tensor("srcb", [ROWS, HID], F32R, kind="ExternalInput")
    mask4_d = nc.dram_tensor("mask4", [EC, E], F32, kind="ExternalInput")
    mif4_d = nc.dram_tensor("mif4", [EC, ROWS], F32, kind="ExternalInput")
    w1ed_d = nc.dram_tensor("w1ed", [EC + NCTX, HID], F32R, kind="ExternalInput")
    b2c_d = nc.dram_tensor("b2c", [128, 2], F32, kind="ExternalInput")
    b3_d = nc.dram_tensor("b3", [EC], F32, kind="ExternalInput")
    w2_d = nc.dram_tensor("w2", [HID, HID], F32R, kind="ExternalInput")
    w3_d = nc.dram_tensor("w3", [HID, EC], F32R, kind="ExternalInput")
    rhsstat_d = nc.dram_tensor("rhsstat", [NCTX + RPT, E], F32R, kind="ExternalInput")
    out_d = nc.dram_tensor("out", [NT, EC, E], F32, kind="ExternalOutput")

    with tile.TileContext(nc) as tc:
        with tc.tile_pool(name="const", bufs=1) as cp, \
             tc.tile_pool(name="h", bufs=2) as hp, \
             tc.tile_pool(name="o", bufs=3) as op, \
             tc.tile_pool(name="ps", bufs=1, space="PSUM") as pp:

            # ---------- augmented layer-1 operands (ping-pong pairs) ----------
            # lh[q]: [KAUG, 256] stationary tile, halves at cols 0:128 / 128:256.
            # rhs_t[q]: [KAUG, E] moving tile.  These feed tile 0, so their
            # loads come first and are merged into one DMA per buffer.
            lh = [None, None]
            rhs_t = [None, None]
            for q in range(2):
                lt = cp.tile([KAUG, HID], F32R, tag=f"lh{q}")
                nc.sync.dma_start(out=lt[0:EC + NCTX, :], in_=w1ed_d[:])
                lh[q] = lt
                rt = cp.tile([KAUG, E], F32R, tag=f"rhs{q}")
                nc.sync.dma_start(out=rt[EC:KAUG, :], in_=rhsstat_d[:])
                rhs_t[q] = rt

            # srcbT[i, h] = node_ctx[i] @ W1s + temb @ W1t + b1 (host-prepared)
            srcbT = cp.tile([ROWS, HID], F32R, tag="srcbT")
            nc.gpsimd.dma_start(out=srcbT, in_=srcb_d[:])

            # ---------- constant loads (Pool/SWDGE: off the HWDGE queue) ----------
            w2k0 = cp.tile([128, HID], F32R, tag="w2k0")
            nc.gpsimd.dma_start(out=w2k0, in_=w2_d[0:128])
            w2k1 = cp.tile([128, HID], F32R, tag="w2k1")
            nc.gpsimd.dma_start(out=w2k1, in_=w2_d[128:256])
            w30 = cp.tile([128, EC], F32R, tag="w30")
            nc.gpsimd.dma_start(out=w30, in_=w3_d[0:128])
            w31 = cp.tile([128, EC], F32R, tag="w31")
            nc.gpsimd.dma_start(out=w31, in_=w3_d[128:256])
            b2c = cp.tile([128, 2], F32, tag="b2c")
            nc.gpsimd.dma_start(out=b2c, in_=b2c_d[:])
            b3c = cp.tile([EC, 1], F32, tag="b3c")
            nc.gpsimd.dma_start(out=b3c, in_=b3_d[:].rearrange("(p o) -> p o", o=1))

            # mask tiles (host-prepared): mask4[c, r*N+j] = mask_j; mif4[c, i] = mask_i
            mask4 = cp.tile([EC, E], F32, tag="mask4")
            nc.sync.dma_start(out=mask4, in_=mask4_d[:])
            mif4 = cp.tile([EC, ROWS], F32, tag="mif4")
            nc.sync.dma_start(out=mif4, in_=mif4_d[:])

            # ---------- main loop: 3-stage software pipeline ----------
            h1s, h2s, ots = {}, {}, {}
            for k in range(NT + 2):
                if k < NT:
                    rhs = rhs_t[k % 2]
                    nc.sync.dma_start(out=rhs[0:EC, :], in_=edge_d[k])
                    lht = lh[k % 2]
                    # per-tile srcbias rows -> stationary tile (SWDGE, Pool)
                    nc.gpsimd.dma_start(out=lht[EC + NCTX:KAUG, :],
                                        in_=srcbT[RPT * k:RPT * (k + 1), :])
                    h1 = hp.tile([128, 2 * E], F32R, tag="h1")
                    for h in range(2):
                        p1h = pp.tile([128, E], F32, name=f"p1_{k}_{h}",
                                      tag=f"p1{'ab'[h]}")
                        for c in range(NCH):
                            nc.tensor.matmul(
                                p1h[:, c * CH:(c + 1) * CH],
                                lhsT=lht[:, h * 128:(h + 1) * 128],
                                rhs=rhs[:, c * CH:(c + 1) * CH],
                                start=True, stop=True)
                        nc.scalar.activation(h1[:, h * E:(h + 1) * E], p1h, AF.Silu)
                    h1s[k] = h1

                if 1 <= k <= NT:
                    j = k - 1
                    h1 = h1s.pop(j)
                    h2 = hp.tile([128, 2 * E], F32R, tag="h2")
                    for h in range(2):
                        # per-half PSUM tile: silu2 of half a releases its slot
                        # before half b finishes, loosening the L2(j+1) gate
                        p2h = pp.tile([128, E], F32, name=f"p2_{j}_{h}",
                                      tag=f"p2{'ab'[h]}")
                        for c in range(NCH):
                            dst = p2h[:, c * CH:(c + 1) * CH]
                            nc.tensor.matmul(
                                dst, lhsT=w2k0[:, h * 128:(h + 1) * 128],
                                rhs=h1[:, c * CH:(c + 1) * CH],
                                start=True, stop=False)
                            nc.tensor.matmul(
                                dst, lhsT=w2k1[:, h * 128:(h + 1) * 128],
                                rhs=h1[:, E + c * CH:E + (c + 1) * CH],
                                start=False, stop=True)
                        nc.scalar.activation(h2[:, h * E:(h + 1) * E], p2h, AF.Silu,
                                             bias=b2c[:, h:h + 1])
                    h2s[j] = h2

                if k >= 2:
                    i = k - 2
                    h2 = h2s.pop(i)
                    p3 = pp.tile([EC, E], F32, name=f"p3_{i}", tag="p1a")
                    for c in range(NCH):
                        dst = p3[:, c * CH:(c + 1) * CH]
                        nc.tensor.matmul(dst, lhsT=w30,
                                         rhs=h2[:, c * CH:(c + 1) * CH],
                                         start=True, stop=False)
                        nc.tensor.matmul(dst, lhsT=w31,
                                         rhs=h2[:, E + c * CH:E + (c + 1) * CH],
                                         start=False, stop=True)
                    ot = op.tile([EC, E], F32, name=f"ot{i}", tag="ot")
                    # one big op reads PSUM (releases the p13 slot early)
                    nc.vector.tensor_scalar_add(out=ot, in0=p3, scalar1=b3c)
                    nc.vector.tensor_mul(out=ot, in0=ot, in1=mask4)  # * mask_j
                    for r in range(RPT):
                        # * mask_i for this grid row
                        nc.vector.tensor_scalar_mul(
                            out=ot[:, r * N:(r + 1) * N],
                            in0=ot[:, r * N:(r + 1) * N],
                            scalar1=mif4[:, RPT * i + r:RPT * i + r + 1])
                    nc.sync.dma_start(out=out_d[i], in_=ot)

    nc.compile()
    return nc


def _get_nc():
    if "nc" not in _CACHE:
        _CACHE["nc"] = _build()
    return _CACHE["nc"]


def _time_embedding(t):
    half = TDIM // 2
    freqs = np.exp(-np.arange(half, dtype=np.float32)
                   * (np.float32(np.log(10000.0)) / np.float32(half - 1)))
    args = np.asarray(t).astype(np.float32)[:, None] * freqs[None, :]
    return np.concatenate([np.sin(args), np.cos(args)], axis=1).astype(np.float32)


def _indicator():
    ind = np.zeros((RPT, E), dtype=np.float32)
    for r in range(RPT):
        ind[r, r * N:(r + 1) * N] = 1.0
    return ind


def _prepare_in_maps(edge_t, x_cond, code_cond, t, node_mask, W1, b1, W2, b2, W3, b3):
    edge_t = np.ascontiguousarray(np.asarray(edge_t, dtype=np.float32))
    node_ctx = np.concatenate(
        [np.asarray(x_cond, np.float32), np.asarray(code_cond, np.float32)], axis=-1)
    temb = _time_embedding(t)                       # [B, TDIM]
    maskf = np.asarray(node_mask).astype(np.float32)  # [B, N]
    W1 = np.asarray(W1, np.float32)
    w1e = np.ascontiguousarray(W1[0:EC])
    w1s = W1[EC:EC + NCTX]
    w1d = np.ascontiguousarray(W1[EC + NCTX:EC + 2 * NCTX])
    w1t = W1[EC + 2 * NCTX:]
    b1 = np.asarray(b1, np.float32)
    b2c = np.ascontiguousarray(np.asarray(b2, np.float32).reshape(2, 128).T)
    b3 = np.asarray(b3, np.float32)
    W2 = np.ascontiguousarray(np.asarray(W2, np.float32))
    W3 = np.ascontiguousarray(np.asarray(W3, np.float32))
    # srcbias (bias precomputation — 0.1% of model FLOPs): [B*N, HID]
    srcb_full = (node_ctx.reshape(B * N, NCTX) @ w1s
                 + (temb @ w1t + b1)[:, None, :].repeat(N, axis=1).reshape(B * N, HID)
                 ).astype(np.float32)

    in_maps = []
    for c in range(NCORES):
        b, ih = c // 2, c % 2
        i0 = ih * ROWS
        es = edge_t[b, i0:i0 + ROWS]               # [ROWS, N, EC]
        er = np.ascontiguousarray(
            es.reshape(NT, RPT, N, EC).transpose(0, 3, 1, 2).reshape(NT, EC, E))
        in_maps.append({
            "edge": er,
            "nctxT": np.ascontiguousarray(node_ctx[b].T),
            "srcb": np.ascontiguousarray(srcb_full[b * N + i0:b * N + i0 + ROWS]),
            "mask4": np.ascontiguousarray(np.tile(maskf[b], (EC, RPT))),
            "mif4": np.ascontiguousarray(
                np.tile(maskf[b, i0:i0 + ROWS], (EC, 1))),
            "w1ed": np.ascontiguousarray(np.vstack([w1e, w1d])),
            "b2c": b2c, "b3": b3, "w2": W2, "w3": W3,
            "rhsstat": np.ascontiguousarray(
                np.vstack([np.tile(node_ctx[b].T, (1, RPT)), _indicator()])),
        })
    return in_maps


def _assemble(results):
    out = np.empty((B, N, N, EC), dtype=np.float32)
    for c in range(NCORES):
        b, ih = c // 2, c % 2
        i0 = ih * ROWS
        o = results[c]["out"]                      # [NT, EC, E]
        out[b, i0:i0 + ROWS] = (
            o.reshape(NT, EC, RPT, N).transpose(0, 2, 3, 1).reshape(ROWS, N, EC))
    return out


def _run(in_maps, trace=False, **kwargs):
    nc = _get_nc()
    return run_bass_kernel_spmd(nc, in_maps, list(range(NCORES)), trace=trace, **kwargs)


def kernel(**inputs):
    in_maps = _prepare_in_maps(**inputs)
    res = _run(in_maps)
    return _assemble(res.results)
